# revision 55
# baseline (speedup 1.0000x reference)
"""BERT input representation kernel for 8 TRN2 NeuronCores.

Math (reference):
    x1  = x @ W_emb + b_emb                      # [B,S,D]
    seg = einsum('bnsd,s->bnd', x1.reshape(B,S/8,8,D), w_seg) + b_seg
    out = (x1.reshape(...) + seg[:,:,None,:]).reshape(B,S,D) + PE(S,D)

Folded form used here (exact algebra):
    out[b,s,:] = (M @ x[b])[s,:] @ W_emb + bias[s,:]
where M = I + blockdiag(ones(8,1) @ w_seg[None,:]) mixes rows within each
8-row segment, and bias[s,:] = PE[s,:] + b_emb*(1 + sum(w_seg)) + b_seg.

Key optimization: the bias matrix, viewed per 128-row tile (4 distinct
tiles, s-period 512), factors as bias_tile[tb] = g @ V_tb with a SHARED
within-tile basis g [128, 64] — numerical rank of the [128, 4*1024]
reshape is ~48 (sinusoidal PE splits into tile-phase x within-tile
sinusoids). So the bias rides the main matmul as 64 extra K-rows
(TensorE cost is N-dependent, not K-dependent), the PSUM drain becomes a
pure copy (no tensor_tensor add), and the copies split across DVE and
ACT.  Output is stored bf16 (tolerance 2e-2 >> bf16 rounding 1.7e-3) and
upcast to f32 on the host, halving the dominant HBM store traffic.

Per core (8 batches = 4096 rows = 32 tiles of 128):
  - transpose-mix: per tile, matmul(lhsT=x chunk [128,64], rhs=M^T
    [128,128]) -> PSUM [64,128] = (M x)^T; ACT drains <=4-tile chunks
    into lhs_sb partitions 0-63 (bf16); chunks are emitted one ahead of
    the consuming tiles so the drain clears ACT's in-order queue before
    the PE needs the result
  - lhs_sb bottom partitions 64-127 hold g^T, replicated to all 32 tile
    column blocks by DVE doubling copies (one 16 KiB DMA seed), emitted
    just-in-time so same-tile write chains in the dep tracker never
    stall an early transpose drain behind a late rep
  - mains: per tile, 2 matmuls K=128 N=512: lhsT = lhs_sb block
    ([xt ; g^T]), rhs = wv block ([W ; V_tb]); W is replicated on-chip
    (DVE) to pair with the 4 V blocks, V loads once per block
  - drain: one engine takes BOTH [128,512] halves of a tile
    (single-writer o_tile, each store joins one engine); tiles
    alternate DVE/ACT with DVE also covering t%8==7 (the tile before
    every other chunk boundary, freeing ACT for transpose drains)
  - per-tile 256 KiB stores, mostly on the sync ring (a dma_start on
    nc.scalar occupies the ACT sequencer through ring lock-wait +
    descriptor gen, starving drains), every 4th on scalar to relieve
    the ring
"""

import sys

if "/opt/trn_rl_repo" not in sys.path:
    sys.path.insert(0, "/opt/trn_rl_repo")

import ml_dtypes
import numpy as np

import concourse.bacc as bacc
import concourse.mybir as mybir
import concourse.tile as tile
from concourse.bass_utils import run_bass_kernel_spmd

B, S, F, D, SEG = 64, 512, 64, 1024, 8
N_CORES = 8
B_LOC = B // N_CORES          # batches per core
ROWS = B_LOC * S              # 4096 rows per core
TILE_P = 128                  # rows per tile
N_TILES = ROWS // TILE_P      # 32
N_PAIR = N_TILES // 2         # 16
N_BIAS = S // TILE_P          # 4 distinct bias row-tiles
RNK = 48                      # bias factorization rank
HD = D // 2                   # 512


_NC_CACHE = None


def _build_nc():
    nc = bacc.Bacc("TRN2", target_bir_lowering=False, debug=False,
                   num_devices=N_CORES)
    # xc: cols [0:128] = M^T, then x rearranged [128, 32*64]
    # (xr[p, t*F:(t+1)*F] = x[t*128+p]), all bf16
    xc_d = nc.declare_dram_parameter("xc", [TILE_P, TILE_P + N_TILES * F],
                                     mybir.dt.bfloat16, isOutput=False)
    # cc: [64, 128+1024+4096] = [g^T (48 rows) | W | V (48 rows)]
    cc_d = nc.declare_dram_parameter("cc", [64, TILE_P + D + N_BIAS * D],
                                     mybir.dt.bfloat16, isOutput=False)
    # out: tile layout [128, 32*1024]; block t = tile t, partition q =
    # row q within the tile; host unscrambles + upcasts
    out_d = nc.declare_dram_parameter("out", [TILE_P, N_TILES * D],
                                      mybir.dt.bfloat16, isOutput=True)

    with tile.TileContext(nc) as tc:
        with (
            tc.tile_pool(name="const", bufs=1) as cpool,
            tc.tile_pool(name="outp", bufs=8) as opool,
            tc.tile_pool(name="ps_t", bufs=1, space="PSUM") as pst,
            tc.tile_pool(name="ps_o", bufs=7, space="PSUM") as pso,
        ):
            # lhs_sb block t: partitions 0-63 = (M x)^T features of tile
            # t, partitions 64-127 = g^T (replicated)
            lhs_sb = cpool.tile([TILE_P, N_TILES * TILE_P],
                                mybir.dt.bfloat16)
            # wv block tb: partitions 0-63 = W, 64-127 = V_tb
            wv_sb = cpool.tile([TILE_P, N_BIAS * D], mybir.dt.bfloat16)
            # M^T and x in one tile so the first load fuses both into
            # 2.25 KiB-per-partition descriptors
            atx = cpool.tile([TILE_P, TILE_P + N_TILES * F],
                             mybir.dt.bfloat16)
            at_ap = atx[:, 0:TILE_P]
            x_sb = atx[:, TILE_P:TILE_P + N_TILES * F]

            # sync ring: g seed (tiny), M^T + chunk-0 x (small, so the
            # first transposes start early), then the rest of x
            nc.sync.dma_start(lhs_sb[64:64 + RNK, 0:TILE_P],
                              cc_d[0:RNK, 0:TILE_P])
            nc.sync.dma_start(atx[:, 0:TILE_P + 128],
                              xc_d[:, 0:TILE_P + 128])
            nc.sync.dma_start(atx[:, TILE_P + 128:TILE_P + 1024],
                              xc_d[:, TILE_P + 128:TILE_P + 1024])
            nc.sync.dma_start(atx[:, TILE_P + 1024:TILE_P + 2048],
                              xc_d[:, TILE_P + 1024:TILE_P + 2048])
            # scalar ring: W + V per-block (tile 0's constants first)
            nc.scalar.dma_start(wv_sb[0:64, 0:D],
                                cc_d[:, TILE_P:TILE_P + D])
            for vb in range(N_BIAS):
                nc.scalar.dma_start(
                    wv_sb[64:64 + RNK, vb * D:(vb + 1) * D],
                    cc_d[0:RNK,
                         TILE_P + (vb + 1) * D:TILE_P + (vb + 2) * D])

            CHUNKS = [1, 1, 2, 4, 4, 4, 4, 4, 4, 4]  # tiles per chunk
            starts = [sum(CHUNKS[:i]) for i in range(len(CHUNKS))]

            def emit_chunk(ci):
                # transpose-mix for chunk ci into one PSUM bank; drain
                # mostly on ACT (two mid chunks on DVE for balance)
                t0, k = starts[ci], CHUNKS[ci]
                ps_x = pst.tile([64, 512], mybir.dt.float32,
                                name="ps_x", tag="ps_x")
                for i in range(k):
                    nc.tensor.matmul(ps_x[0:64, 128 * i:128 * (i + 1)],
                                     x_sb[:, (t0 + i) * F:(t0 + i + 1) * F],
                                     at_ap, start=True, stop=True)
                dst = lhs_sb[0:64, 128 * t0:128 * (t0 + k)]
                nc.scalar.copy(dst, ps_x[0:64, 0:128 * k])

            # on-chip replication via SBUF->SBUF DMAs (the DMA engines
            # have slack; keeps DVE/ACT free for drains): g^T doubling
            # across the 32 lhs blocks, W across the 4 wv blocks.
            # Emission is interleaved into the tile loop below (just
            # before the chunk that first needs each block) so the dep
            # tracker's same-tile write chains never stall an early
            # transpose drain behind a late rep.
            REPS = {  # keyed by the chunk index that first needs them
                0: [(lhs_sb, 64, RNK, TILE_P, 2 * TILE_P, TILE_P),
                    (wv_sb, 0, 64, D, 2 * D, D)],
                1: [(lhs_sb, 64, RNK, 2 * TILE_P, 4 * TILE_P, 2 * TILE_P),
                    (wv_sb, 0, 64, 2 * D, 4 * D, 2 * D)],
                2: [(lhs_sb, 64, RNK, 4 * TILE_P, 8 * TILE_P, 4 * TILE_P)],
                3: [(lhs_sb, 64, RNK, 8 * TILE_P, 16 * TILE_P, 8 * TILE_P)],
                5: [(lhs_sb, 64, RNK, 16 * TILE_P, 32 * TILE_P,
                     16 * TILE_P)],
            }

            def emit_reps(t):
                for (tile_, p0, ph, c0, c1, w) in REPS.pop(t, ()):
                    nc.vector.tensor_copy(tile_[p0:p0 + ph, c0:c1],
                                          tile_[p0:p0 + ph, c0 - w:c1 - w])

            emit_reps(0)
            emit_chunk(0)
            for ci, k in enumerate(CHUNKS):
                for i in range(k):
                    t = starts[ci] + i
                    tb = t % N_BIAS
                    lhsT = lhs_sb[0:64 + RNK, 128 * t:128 * (t + 1)]
                    o_tile = opool.tile([TILE_P, D], mybir.dt.bfloat16,
                                        name="o_tile", tag="ot")
                    for h in range(2):
                        ps = pso.tile([TILE_P, HD], mybir.dt.float32,
                                      name="ps", tag="ps")
                        nc.tensor.matmul(
                            ps[:], lhsT,
                            wv_sb[0:64 + RNK,
                                  tb * D + h * HD:tb * D + (h + 1) * HD],
                            start=True, stop=True)
                        # half-tile drains, one-bank slots; one engine
                        # takes BOTH halves of a tile (single-writer
                        # o_tile, store joins one engine), tiles
                        # alternate; t%8==7 goes to DVE for balance
                        if t % 2 == 0 or t % 8 == 7:
                            nc.vector.tensor_copy(
                                o_tile[:, h * HD:(h + 1) * HD], ps[:])
                        else:
                            nc.scalar.copy(
                                o_tile[:, h * HD:(h + 1) * HD], ps[:])
                    # per-tile store (2 KiB lines cost the same per byte
                    # as 4 KiB; finer granularity shortens the tail);
                    # mostly on sync (scalar-ring stores occupy the ACT
                    # sequencer), a few on scalar to relieve the ring
                    eng = nc.scalar if t % 4 == 3 else nc.sync
                    eng.dma_start(out_d[:, D * t:D * (t + 1)],
                                  o_tile[:])
                    # software pipeline: transpose chunks are emitted
                    # TWO chunks ahead so their PSUM drain clears the
                    # in-order drain-engine backlog well before the PE
                    # needs the result (avoids chunk-boundary PE stalls)
                    if i == min(1, k - 1) and ci + 1 < len(CHUNKS):
                        emit_reps(ci + 1)
                        emit_chunk(ci + 1)
    nc.compile()
    return nc


def _host_constants(W_emb, b_emb, w_seg, b_seg):
    # sinusoidal positional encoding, float32, same formula as reference
    pos = np.arange(S, dtype=np.float32)[:, None]
    div = np.exp(np.arange(0, D, 2, dtype=np.float32)
                 * (-np.log(10000.0) / D)).astype(np.float32)
    ang = pos * div
    pe = np.zeros((S, D), np.float32)
    pe[:, 0::2] = np.sin(ang)
    pe[:, 1::2] = np.cos(ang)

    bias = (pe + b_emb[None, :] * (np.float32(1.0) + w_seg.sum())
            + b_seg[0]).astype(np.float64)
    # within-tile factorization: bias.reshape(4,128,D) -> [128, 4*D],
    # rank-64 SVD; V re-solved against the bf16-quantized g
    B_all = bias.reshape(N_BIAS, TILE_P, D).transpose(1, 0, 2).reshape(
        TILE_P, N_BIAS * D)
    U, sv, Vt = np.linalg.svd(B_all, full_matrices=False)
    g = (U[:, :RNK] * np.sqrt(sv[:RNK])).astype(
        ml_dtypes.bfloat16).astype(np.float64)
    V, *_ = np.linalg.lstsq(g, B_all, rcond=None)
    gT = np.zeros((64, TILE_P), ml_dtypes.bfloat16)
    gT[:RNK] = np.ascontiguousarray(g.T).astype(ml_dtypes.bfloat16)
    Vb = np.zeros((64, N_BIAS * D), ml_dtypes.bfloat16)
    Vb[:RNK] = np.ascontiguousarray(V).astype(ml_dtypes.bfloat16)

    # M^T[p, n] = delta + w_seg[p % 8] within each 8-row segment
    blk = np.eye(SEG, dtype=np.float32) + w_seg[:, None] * np.ones(
        (1, SEG), np.float32)
    at = np.kron(np.eye(TILE_P // SEG, dtype=np.float32), blk).astype(
        ml_dtypes.bfloat16)

    Wb = W_emb.astype(ml_dtypes.bfloat16)                      # [64, 1024]
    cc = np.ascontiguousarray(np.concatenate([gT, Wb, Vb], axis=1))
    return at, cc


def _prepare_in_maps(x, W_emb, b_emb, w_seg, b_seg):
    x = np.ascontiguousarray(np.asarray(x, dtype=np.float32))
    W_emb = np.asarray(W_emb, dtype=np.float32)
    b_emb = np.asarray(b_emb, dtype=np.float32)
    w_seg = np.asarray(w_seg, dtype=np.float32)
    b_seg = np.asarray(b_seg, dtype=np.float32)

    at, cc = _host_constants(W_emb, b_emb, w_seg, b_seg)

    in_maps = []
    for c in range(N_CORES):
        xs = x[c * B_LOC:(c + 1) * B_LOC].reshape(ROWS, F)
        xr = np.ascontiguousarray(
            xs.reshape(N_TILES, TILE_P, F).transpose(1, 0, 2).reshape(
                TILE_P, N_TILES * F)).astype(ml_dtypes.bfloat16)
        in_maps.append(
            {"xc": np.ascontiguousarray(np.concatenate([at, xr], axis=1)),
             "cc": cc})
    return in_maps


def kernel(x, W_emb, b_emb, w_seg, b_seg):
    in_maps = _prepare_in_maps(x, W_emb, b_emb, w_seg, b_seg)

    global _NC_CACHE
    if _NC_CACHE is None:
        _NC_CACHE = _build_nc()

    res = run_bass_kernel_spmd(_NC_CACHE, in_maps,
                               core_ids=list(range(N_CORES)))
    out = np.concatenate(
        [np.asarray(res.results[c]["out"])
         .reshape(TILE_P, N_TILES, D).transpose(1, 0, 2)
         .reshape(B_LOC, S, D).astype(np.float32)
         for c in range(N_CORES)], axis=0)
    return out


# revision 56
# speedup vs baseline: 1.0882x; 1.0882x over previous
"""BERT input representation kernel for 8 TRN2 NeuronCores.

Math (reference):
    x1  = x @ W_emb + b_emb                      # [B,S,D]
    seg = einsum('bnsd,s->bnd', x1.reshape(B,S/8,8,D), w_seg) + b_seg
    out = (x1.reshape(...) + seg[:,:,None,:]).reshape(B,S,D) + PE(S,D)

Folded form used here (exact algebra):
    out[b,s,:] = (M @ x[b])[s,:] @ W_emb + bias[s,:]
where M = I + blockdiag(ones(8,1) @ w_seg[None,:]) mixes rows within each
8-row segment, and bias[s,:] = PE[s,:] + b_emb*(1 + sum(w_seg)) + b_seg.

Key optimization: the bias matrix, viewed per 128-row tile (4 distinct
tiles, s-period 512), factors as bias_tile[tb] = g @ V_tb with a SHARED
within-tile basis g [128, 64] — numerical rank of the [128, 4*1024]
reshape is ~48 (sinusoidal PE splits into tile-phase x within-tile
sinusoids). So the bias rides the main matmul as 64 extra K-rows
(TensorE cost is N-dependent, not K-dependent), the PSUM drain becomes a
pure copy (no tensor_tensor add), and the copies split across DVE and
ACT.  Output is stored bf16 (tolerance 2e-2 >> bf16 rounding 1.7e-3) and
upcast to f32 on the host, halving the dominant HBM store traffic.

Per core (8 batches = 4096 rows = 32 tiles of 128):
  - transpose-mix: per tile, matmul(lhsT=x chunk [128,64], rhs=M^T
    [128,128]) -> PSUM [64,128] = (M x)^T; ACT drains <=4-tile chunks
    into lhs_sb partitions 0-63 (bf16); chunks are emitted one ahead of
    the consuming tiles so the drain clears ACT's in-order queue before
    the PE needs the result
  - lhs_sb bottom partitions 64-127 hold g^T, replicated to all 32 tile
    column blocks by DVE doubling copies (one 16 KiB DMA seed), emitted
    just-in-time so same-tile write chains in the dep tracker never
    stall an early transpose drain behind a late rep
  - mains: per tile, 2 matmuls K=128 N=512: lhsT = lhs_sb block
    ([xt ; g^T]), rhs = wv block ([W ; V_tb]); W is replicated on-chip
    (DVE) to pair with the 4 V blocks, V loads once per block
  - drain: one engine takes BOTH [128,512] halves of a tile
    (single-writer o_tile, each store joins one engine); tiles
    alternate DVE/ACT with DVE also covering t%8==7 (the tile before
    every other chunk boundary, freeing ACT for transpose drains)
  - per-tile 256 KiB stores, mostly on the sync ring (a dma_start on
    nc.scalar occupies the ACT sequencer through ring lock-wait +
    descriptor gen, starving drains), every 4th on scalar to relieve
    the ring
"""

import sys

if "/opt/trn_rl_repo" not in sys.path:
    sys.path.insert(0, "/opt/trn_rl_repo")

import ml_dtypes
import numpy as np

import concourse.bacc as bacc
import concourse.mybir as mybir
import concourse.tile as tile
from concourse.bass_utils import run_bass_kernel_spmd

B, S, F, D, SEG = 64, 512, 64, 1024, 8
N_CORES = 8
B_LOC = B // N_CORES          # batches per core
ROWS = B_LOC * S              # 4096 rows per core
TILE_P = 128                  # rows per tile
N_TILES = ROWS // TILE_P      # 32
N_PAIR = N_TILES // 2         # 16
N_BIAS = S // TILE_P          # 4 distinct bias row-tiles
RNK = 48                      # bias factorization rank
HD = D // 2                   # 512


_NC_CACHE = None


def _build_nc():
    nc = bacc.Bacc("TRN2", target_bir_lowering=False, debug=False,
                   num_devices=N_CORES)
    # xc: cols [0:128] = M^T, then x rearranged [128, 32*64]
    # (xr[p, t*F:(t+1)*F] = x[t*128+p]), all bf16
    xc_d = nc.declare_dram_parameter("xc", [TILE_P, TILE_P + N_TILES * F],
                                     mybir.dt.bfloat16, isOutput=False)
    # cc: [64, 128+1024+4096] = [g^T (48 rows) | W | V (48 rows)]
    cc_d = nc.declare_dram_parameter("cc", [64, TILE_P + D + N_BIAS * D],
                                     mybir.dt.bfloat16, isOutput=False)
    # out: tile layout [128, 32*1024]; block t = tile t, partition q =
    # row q within the tile; host unscrambles + upcasts
    out_d = nc.declare_dram_parameter("out", [TILE_P, N_TILES * D],
                                      mybir.dt.bfloat16, isOutput=True)

    with tile.TileContext(nc) as tc:
        with (
            tc.tile_pool(name="const", bufs=1) as cpool,
            tc.tile_pool(name="outp", bufs=8) as opool,
            tc.tile_pool(name="ps_t", bufs=2, space="PSUM") as pst,
            tc.tile_pool(name="ps_o", bufs=6, space="PSUM") as pso,
        ):
            # lhs_sb block t: partitions 0-63 = (M x)^T features of tile
            # t, partitions 64-127 = g^T (replicated)
            lhs_sb = cpool.tile([TILE_P, N_TILES * TILE_P],
                                mybir.dt.bfloat16)
            # wv block tb: partitions 0-63 = W, 64-127 = V_tb
            wv_sb = cpool.tile([TILE_P, N_BIAS * D], mybir.dt.bfloat16)
            # M^T and x in one tile so the first load fuses both into
            # 2.25 KiB-per-partition descriptors
            atx = cpool.tile([TILE_P, TILE_P + N_TILES * F],
                             mybir.dt.bfloat16)
            at_ap = atx[:, 0:TILE_P]
            x_sb = atx[:, TILE_P:TILE_P + N_TILES * F]

            # sync ring: g seed (tiny), M^T + chunk-0 x (small, so the
            # first transposes start early), then the rest of x
            nc.sync.dma_start(lhs_sb[64:64 + RNK, 0:TILE_P],
                              cc_d[0:RNK, 0:TILE_P])
            nc.sync.dma_start(atx[:, 0:TILE_P + 128],
                              xc_d[:, 0:TILE_P + 128])
            nc.sync.dma_start(atx[:, TILE_P + 128:TILE_P + 1024],
                              xc_d[:, TILE_P + 128:TILE_P + 1024])
            nc.sync.dma_start(atx[:, TILE_P + 1024:TILE_P + 2048],
                              xc_d[:, TILE_P + 1024:TILE_P + 2048])
            # scalar ring: W + V per-block (tile 0's constants first)
            nc.scalar.dma_start(wv_sb[0:64, 0:D],
                                cc_d[:, TILE_P:TILE_P + D])
            for vb in range(N_BIAS):
                nc.scalar.dma_start(
                    wv_sb[64:64 + RNK, vb * D:(vb + 1) * D],
                    cc_d[0:RNK,
                         TILE_P + (vb + 1) * D:TILE_P + (vb + 2) * D])

            CHUNKS = [1, 1, 2, 4, 4, 4, 4, 4, 4, 4]  # tiles per chunk
            starts = [sum(CHUNKS[:i]) for i in range(len(CHUNKS))]

            def emit_chunk(ci):
                # transpose-mix for chunk ci into one PSUM bank; drain
                # mostly on ACT (two mid chunks on DVE for balance)
                t0, k = starts[ci], CHUNKS[ci]
                ps_x = pst.tile([64, 512], mybir.dt.float32,
                                name="ps_x", tag="ps_x")
                for i in range(k):
                    nc.tensor.matmul(ps_x[0:64, 128 * i:128 * (i + 1)],
                                     x_sb[:, (t0 + i) * F:(t0 + i + 1) * F],
                                     at_ap, start=True, stop=True)
                dst = lhs_sb[0:64, 128 * t0:128 * (t0 + k)]
                nc.scalar.copy(dst, ps_x[0:64, 0:128 * k])

            # on-chip replication via SBUF->SBUF DMAs (the DMA engines
            # have slack; keeps DVE/ACT free for drains): g^T doubling
            # across the 32 lhs blocks, W across the 4 wv blocks.
            # Emission is interleaved into the tile loop below (just
            # before the chunk that first needs each block) so the dep
            # tracker's same-tile write chains never stall an early
            # transpose drain behind a late rep.
            REPS = {  # keyed by the chunk index that first needs them
                0: [(lhs_sb, 64, RNK, TILE_P, 2 * TILE_P, TILE_P),
                    (wv_sb, 0, 64, D, 2 * D, D)],
                1: [(lhs_sb, 64, RNK, 2 * TILE_P, 4 * TILE_P, 2 * TILE_P),
                    (wv_sb, 0, 64, 2 * D, 4 * D, 2 * D)],
                2: [(lhs_sb, 64, RNK, 4 * TILE_P, 8 * TILE_P, 4 * TILE_P)],
                3: [(lhs_sb, 64, RNK, 8 * TILE_P, 16 * TILE_P, 8 * TILE_P)],
                5: [(lhs_sb, 64, RNK, 16 * TILE_P, 32 * TILE_P,
                     16 * TILE_P)],
            }

            def emit_reps(t):
                for (tile_, p0, ph, c0, c1, w) in REPS.pop(t, ()):
                    nc.vector.tensor_copy(tile_[p0:p0 + ph, c0:c1],
                                          tile_[p0:p0 + ph, c0 - w:c1 - w])

            emit_reps(0)
            emit_chunk(0)
            for ci, k in enumerate(CHUNKS):
                for i in range(k):
                    t = starts[ci] + i
                    tb = t % N_BIAS
                    lhsT = lhs_sb[0:64 + RNK, 128 * t:128 * (t + 1)]
                    o_tile = opool.tile([TILE_P, D], mybir.dt.bfloat16,
                                        name="o_tile", tag="ot")
                    for h in range(2):
                        ps = pso.tile([TILE_P, HD], mybir.dt.float32,
                                      name="ps", tag="ps")
                        nc.tensor.matmul(
                            ps[:], lhsT,
                            wv_sb[0:64 + RNK,
                                  tb * D + h * HD:tb * D + (h + 1) * HD],
                            start=True, stop=True)
                        # half-tile drains, one-bank slots; one engine
                        # takes BOTH halves of a tile (single-writer
                        # o_tile, store joins one engine), tiles
                        # alternate; t%8==7 goes to DVE for balance
                        if t % 2 == 0 or t % 8 == 7:
                            nc.vector.tensor_copy(
                                o_tile[:, h * HD:(h + 1) * HD], ps[:])
                        else:
                            nc.scalar.copy(
                                o_tile[:, h * HD:(h + 1) * HD], ps[:])
                    # per-tile store (2 KiB lines cost the same per byte
                    # as 4 KiB; finer granularity shortens the tail);
                    # mostly on sync (scalar-ring stores occupy the ACT
                    # sequencer), a few on scalar to relieve the ring
                    eng = nc.scalar if t % 4 == 3 else nc.sync
                    eng.dma_start(out_d[:, D * t:D * (t + 1)],
                                  o_tile[:])
                    # software pipeline: transpose chunks are emitted
                    # TWO chunks ahead so their PSUM drain clears the
                    # in-order drain-engine backlog well before the PE
                    # needs the result (avoids chunk-boundary PE stalls)
                    if i == min(1, k - 1) and ci + 1 < len(CHUNKS):
                        emit_reps(ci + 1)
                        emit_chunk(ci + 1)
    nc.compile()
    return nc


def _host_constants(W_emb, b_emb, w_seg, b_seg):
    # sinusoidal positional encoding, float32, same formula as reference
    pos = np.arange(S, dtype=np.float32)[:, None]
    div = np.exp(np.arange(0, D, 2, dtype=np.float32)
                 * (-np.log(10000.0) / D)).astype(np.float32)
    ang = pos * div
    pe = np.zeros((S, D), np.float32)
    pe[:, 0::2] = np.sin(ang)
    pe[:, 1::2] = np.cos(ang)

    bias = (pe + b_emb[None, :] * (np.float32(1.0) + w_seg.sum())
            + b_seg[0]).astype(np.float64)
    # within-tile factorization: bias.reshape(4,128,D) -> [128, 4*D],
    # rank-64 SVD; V re-solved against the bf16-quantized g
    B_all = bias.reshape(N_BIAS, TILE_P, D).transpose(1, 0, 2).reshape(
        TILE_P, N_BIAS * D)
    U, sv, Vt = np.linalg.svd(B_all, full_matrices=False)
    g = (U[:, :RNK] * np.sqrt(sv[:RNK])).astype(
        ml_dtypes.bfloat16).astype(np.float64)
    V, *_ = np.linalg.lstsq(g, B_all, rcond=None)
    gT = np.zeros((64, TILE_P), ml_dtypes.bfloat16)
    gT[:RNK] = np.ascontiguousarray(g.T).astype(ml_dtypes.bfloat16)
    Vb = np.zeros((64, N_BIAS * D), ml_dtypes.bfloat16)
    Vb[:RNK] = np.ascontiguousarray(V).astype(ml_dtypes.bfloat16)

    # M^T[p, n] = delta + w_seg[p % 8] within each 8-row segment
    blk = np.eye(SEG, dtype=np.float32) + w_seg[:, None] * np.ones(
        (1, SEG), np.float32)
    at = np.kron(np.eye(TILE_P // SEG, dtype=np.float32), blk).astype(
        ml_dtypes.bfloat16)

    Wb = W_emb.astype(ml_dtypes.bfloat16)                      # [64, 1024]
    cc = np.ascontiguousarray(np.concatenate([gT, Wb, Vb], axis=1))
    return at, cc


def _prepare_in_maps(x, W_emb, b_emb, w_seg, b_seg):
    x = np.ascontiguousarray(np.asarray(x, dtype=np.float32))
    W_emb = np.asarray(W_emb, dtype=np.float32)
    b_emb = np.asarray(b_emb, dtype=np.float32)
    w_seg = np.asarray(w_seg, dtype=np.float32)
    b_seg = np.asarray(b_seg, dtype=np.float32)

    at, cc = _host_constants(W_emb, b_emb, w_seg, b_seg)

    in_maps = []
    for c in range(N_CORES):
        xs = x[c * B_LOC:(c + 1) * B_LOC].reshape(ROWS, F)
        xr = np.ascontiguousarray(
            xs.reshape(N_TILES, TILE_P, F).transpose(1, 0, 2).reshape(
                TILE_P, N_TILES * F)).astype(ml_dtypes.bfloat16)
        in_maps.append(
            {"xc": np.ascontiguousarray(np.concatenate([at, xr], axis=1)),
             "cc": cc})
    return in_maps


def kernel(x, W_emb, b_emb, w_seg, b_seg):
    in_maps = _prepare_in_maps(x, W_emb, b_emb, w_seg, b_seg)

    global _NC_CACHE
    if _NC_CACHE is None:
        _NC_CACHE = _build_nc()

    res = run_bass_kernel_spmd(_NC_CACHE, in_maps,
                               core_ids=list(range(N_CORES)))
    out = np.concatenate(
        [np.asarray(res.results[c]["out"])
         .reshape(TILE_P, N_TILES, D).transpose(1, 0, 2)
         .reshape(B_LOC, S, D).astype(np.float32)
         for c in range(N_CORES)], axis=0)
    return out


# revision 57
# speedup vs baseline: 1.1098x; 1.0198x over previous
"""BERT input representation kernel for 8 TRN2 NeuronCores.

Math (reference):
    x1  = x @ W_emb + b_emb                      # [B,S,D]
    seg = einsum('bnsd,s->bnd', x1.reshape(B,S/8,8,D), w_seg) + b_seg
    out = (x1.reshape(...) + seg[:,:,None,:]).reshape(B,S,D) + PE(S,D)

Folded form used here (exact algebra):
    out[b,s,:] = (M @ x[b])[s,:] @ W_emb + bias[s,:]
where M = I + blockdiag(ones(8,1) @ w_seg[None,:]) mixes rows within each
8-row segment, and bias[s,:] = PE[s,:] + b_emb*(1 + sum(w_seg)) + b_seg.

Key optimization: the bias matrix, viewed per 128-row tile (4 distinct
tiles, s-period 512), factors as bias_tile[tb] = g @ V_tb with a SHARED
within-tile basis g [128, 64] — numerical rank of the [128, 4*1024]
reshape is ~48 (sinusoidal PE splits into tile-phase x within-tile
sinusoids). So the bias rides the main matmul as 64 extra K-rows
(TensorE cost is N-dependent, not K-dependent), the PSUM drain becomes a
pure copy (no tensor_tensor add), and the copies split across DVE and
ACT.  Output is stored bf16 (tolerance 2e-2 >> bf16 rounding 1.7e-3) and
upcast to f32 on the host, halving the dominant HBM store traffic.

Per core (8 batches = 4096 rows = 32 tiles of 128):
  - transpose-mix: per tile, matmul(lhsT=x chunk [128,64], rhs=M^T
    [128,128]) -> PSUM [64,128] = (M x)^T; ACT drains <=4-tile chunks
    into lhs_sb partitions 0-63 (bf16); chunks are emitted one ahead of
    the consuming tiles so the drain clears ACT's in-order queue before
    the PE needs the result
  - lhs_sb bottom partitions 64-127 hold g^T, replicated to all 32 tile
    column blocks by DVE doubling copies (one 16 KiB DMA seed), emitted
    just-in-time so same-tile write chains in the dep tracker never
    stall an early transpose drain behind a late rep
  - mains: per tile, 2 matmuls K=128 N=512: lhsT = lhs_sb block
    ([xt ; g^T]), rhs = wv block ([W ; V_tb]); W is replicated on-chip
    (DVE) to pair with the 4 V blocks, V loads once per block
  - drain: one engine takes BOTH [128,512] halves of a tile
    (single-writer o_tile, each store joins one engine); tiles
    alternate DVE/ACT with DVE also covering t%8==7 (the tile before
    every other chunk boundary, freeing ACT for transpose drains)
  - per-tile 256 KiB stores, mostly on the sync ring (a dma_start on
    nc.scalar occupies the ACT sequencer through ring lock-wait +
    descriptor gen, starving drains), every 4th on scalar to relieve
    the ring
"""

import sys

if "/opt/trn_rl_repo" not in sys.path:
    sys.path.insert(0, "/opt/trn_rl_repo")

import ml_dtypes
import numpy as np

import concourse.bacc as bacc
import concourse.mybir as mybir
import concourse.tile as tile
from concourse.bass_utils import run_bass_kernel_spmd

B, S, F, D, SEG = 64, 512, 64, 1024, 8
N_CORES = 8
B_LOC = B // N_CORES          # batches per core
ROWS = B_LOC * S              # 4096 rows per core
TILE_P = 128                  # rows per tile
N_TILES = ROWS // TILE_P      # 32
N_PAIR = N_TILES // 2         # 16
N_BIAS = S // TILE_P          # 4 distinct bias row-tiles
RNK = 48                      # bias factorization rank
HD = D // 2                   # 512


_NC_CACHE = None


def _build_nc():
    nc = bacc.Bacc("TRN2", target_bir_lowering=False, debug=False,
                   num_devices=N_CORES)
    # xc: cols [0:128] = M^T, then x rearranged [128, 32*64]
    # (xr[p, t*F:(t+1)*F] = x[t*128+p]), all bf16
    xc_d = nc.declare_dram_parameter(
        "xc", [TILE_P, TILE_P + 4 * F + N_TILES * F],
        mybir.dt.bfloat16, isOutput=False)
    # cc: [64, 128+1024+4096] = [g^T (48 rows) | W | V (48 rows)]
    cc_d = nc.declare_dram_parameter("cc", [64, TILE_P + D + N_BIAS * D],
                                     mybir.dt.bfloat16, isOutput=False)
    # out: tile layout [128, 32*1024]; block t = tile t, partition q =
    # row q within the tile; host unscrambles + upcasts
    out_d = nc.declare_dram_parameter("out", [TILE_P, N_TILES * D],
                                      mybir.dt.bfloat16, isOutput=True)

    with tile.TileContext(nc) as tc:
        with (
            tc.tile_pool(name="const", bufs=1) as cpool,
            tc.tile_pool(name="outp", bufs=8) as opool,
            tc.tile_pool(name="ps_t", bufs=2, space="PSUM") as pst,
            tc.tile_pool(name="ps_o", bufs=6, space="PSUM") as pso,
        ):
            # lhs_sb block t: partitions 0-63 = (M x)^T features of tile
            # t, partitions 64-127 = g^T (replicated)
            lhs_sb = cpool.tile([TILE_P, N_TILES * TILE_P],
                                mybir.dt.bfloat16)
            # wv block tb: partitions 0-63 = W, 64-127 = V_tb
            wv_sb = cpool.tile([TILE_P, N_BIAS * D], mybir.dt.bfloat16)
            # M^T + a duplicate of tiles 0-3's x in one SMALL tile:
            # the ramp-critical first transposes depend only on this one
            # 96 KiB load, fully decoupled from the big x loads (coarse
            # same-tile dep tracking would chain them otherwise)
            head = cpool.tile([TILE_P, TILE_P + 4 * F], mybir.dt.bfloat16)
            at_ap = head[:, 0:TILE_P]
            x_sb = cpool.tile([TILE_P, N_TILES * F], mybir.dt.bfloat16)
            XO = TILE_P + 4 * F   # x_full offset in xc_d

            # sync ring: g seed (tiny), head, then the full x
            nc.sync.dma_start(lhs_sb[64:64 + RNK, 0:TILE_P],
                              cc_d[0:RNK, 0:TILE_P])
            nc.sync.dma_start(head[:], xc_d[:, 0:TILE_P + 4 * F])
            nc.sync.dma_start(x_sb[:, 0:1024], xc_d[:, XO:XO + 1024])
            nc.sync.dma_start(x_sb[:, 1024:2048],
                              xc_d[:, XO + 1024:XO + 2048])
            # scalar ring: W + V per-block (tile 0's constants first)
            nc.scalar.dma_start(wv_sb[0:64, 0:D],
                                cc_d[:, TILE_P:TILE_P + D])
            for vb in range(N_BIAS):
                nc.scalar.dma_start(
                    wv_sb[64:64 + RNK, vb * D:(vb + 1) * D],
                    cc_d[0:RNK,
                         TILE_P + (vb + 1) * D:TILE_P + (vb + 2) * D])

            CHUNKS = [1, 1, 2, 4, 4, 4, 4, 4, 4, 4]  # tiles per chunk
            starts = [sum(CHUNKS[:i]) for i in range(len(CHUNKS))]

            def emit_chunk(ci):
                # transpose-mix for chunk ci into one PSUM bank; drain
                # mostly on ACT (two mid chunks on DVE for balance)
                t0, k = starts[ci], CHUNKS[ci]
                ps_x = pst.tile([64, 512], mybir.dt.float32,
                                name="ps_x", tag="ps_x")
                for i in range(k):
                    t = t0 + i
                    xsrc = (head[:, TILE_P + t * F:TILE_P + (t + 1) * F]
                            if t < 4 else
                            x_sb[:, t * F:(t + 1) * F])
                    nc.tensor.matmul(ps_x[0:64, 128 * i:128 * (i + 1)],
                                     xsrc, at_ap, start=True, stop=True)
                dst = lhs_sb[0:64, 128 * t0:128 * (t0 + k)]
                nc.scalar.copy(dst, ps_x[0:64, 0:128 * k])

            # on-chip replication via SBUF->SBUF DMAs (the DMA engines
            # have slack; keeps DVE/ACT free for drains): g^T doubling
            # across the 32 lhs blocks, W across the 4 wv blocks.
            # Emission is interleaved into the tile loop below (just
            # before the chunk that first needs each block) so the dep
            # tracker's same-tile write chains never stall an early
            # transpose drain behind a late rep.
            REPS = {  # keyed by the chunk index that first needs them
                0: [(lhs_sb, 64, RNK, TILE_P, 2 * TILE_P, TILE_P),
                    (wv_sb, 0, 64, D, 2 * D, D)],
                1: [(lhs_sb, 64, RNK, 2 * TILE_P, 4 * TILE_P, 2 * TILE_P),
                    (wv_sb, 0, 64, 2 * D, 4 * D, 2 * D)],
                2: [(lhs_sb, 64, RNK, 4 * TILE_P, 8 * TILE_P, 4 * TILE_P)],
                3: [(lhs_sb, 64, RNK, 8 * TILE_P, 16 * TILE_P, 8 * TILE_P)],
                5: [(lhs_sb, 64, RNK, 16 * TILE_P, 32 * TILE_P,
                     16 * TILE_P)],
            }

            def emit_reps(t):
                for (tile_, p0, ph, c0, c1, w) in REPS.pop(t, ()):
                    nc.vector.tensor_copy(tile_[p0:p0 + ph, c0:c1],
                                          tile_[p0:p0 + ph, c0 - w:c1 - w])

            emit_reps(0)
            emit_chunk(0)
            for ci, k in enumerate(CHUNKS):
                for i in range(k):
                    t = starts[ci] + i
                    tb = t % N_BIAS
                    lhsT = lhs_sb[0:64 + RNK, 128 * t:128 * (t + 1)]
                    o_tile = opool.tile([TILE_P, D], mybir.dt.bfloat16,
                                        name="o_tile", tag="ot")
                    for h in range(2):
                        ps = pso.tile([TILE_P, HD], mybir.dt.float32,
                                      name="ps", tag="ps")
                        nc.tensor.matmul(
                            ps[:], lhsT,
                            wv_sb[0:64 + RNK,
                                  tb * D + h * HD:tb * D + (h + 1) * HD],
                            start=True, stop=True)
                        # half-tile drains, one-bank slots; one engine
                        # takes BOTH halves of a tile (single-writer
                        # o_tile, store joins one engine), tiles
                        # alternate; t%8==7 goes to DVE for balance
                        if t % 2 == 0 or t % 8 == 7:
                            nc.vector.tensor_copy(
                                o_tile[:, h * HD:(h + 1) * HD], ps[:])
                        else:
                            nc.scalar.copy(
                                o_tile[:, h * HD:(h + 1) * HD], ps[:])
                    # per-tile store (2 KiB lines cost the same per byte
                    # as 4 KiB; finer granularity shortens the tail);
                    # mostly on sync (scalar-ring stores occupy the ACT
                    # sequencer), a few on scalar to relieve the ring
                    eng = nc.scalar if t % 4 == 3 else nc.sync
                    eng.dma_start(out_d[:, D * t:D * (t + 1)],
                                  o_tile[:])
                    # software pipeline: transpose chunks are emitted
                    # TWO chunks ahead so their PSUM drain clears the
                    # in-order drain-engine backlog well before the PE
                    # needs the result (avoids chunk-boundary PE stalls)
                    if i == min(1, k - 1) and ci + 1 < len(CHUNKS):
                        emit_reps(ci + 1)
                        emit_chunk(ci + 1)
    nc.compile()
    return nc


def _host_constants(W_emb, b_emb, w_seg, b_seg):
    # sinusoidal positional encoding, float32, same formula as reference
    pos = np.arange(S, dtype=np.float32)[:, None]
    div = np.exp(np.arange(0, D, 2, dtype=np.float32)
                 * (-np.log(10000.0) / D)).astype(np.float32)
    ang = pos * div
    pe = np.zeros((S, D), np.float32)
    pe[:, 0::2] = np.sin(ang)
    pe[:, 1::2] = np.cos(ang)

    bias = (pe + b_emb[None, :] * (np.float32(1.0) + w_seg.sum())
            + b_seg[0]).astype(np.float64)
    # within-tile factorization: bias.reshape(4,128,D) -> [128, 4*D],
    # rank-64 SVD; V re-solved against the bf16-quantized g
    B_all = bias.reshape(N_BIAS, TILE_P, D).transpose(1, 0, 2).reshape(
        TILE_P, N_BIAS * D)
    U, sv, Vt = np.linalg.svd(B_all, full_matrices=False)
    g = (U[:, :RNK] * np.sqrt(sv[:RNK])).astype(
        ml_dtypes.bfloat16).astype(np.float64)
    V, *_ = np.linalg.lstsq(g, B_all, rcond=None)
    gT = np.zeros((64, TILE_P), ml_dtypes.bfloat16)
    gT[:RNK] = np.ascontiguousarray(g.T).astype(ml_dtypes.bfloat16)
    Vb = np.zeros((64, N_BIAS * D), ml_dtypes.bfloat16)
    Vb[:RNK] = np.ascontiguousarray(V).astype(ml_dtypes.bfloat16)

    # M^T[p, n] = delta + w_seg[p % 8] within each 8-row segment
    blk = np.eye(SEG, dtype=np.float32) + w_seg[:, None] * np.ones(
        (1, SEG), np.float32)
    at = np.kron(np.eye(TILE_P // SEG, dtype=np.float32), blk).astype(
        ml_dtypes.bfloat16)

    Wb = W_emb.astype(ml_dtypes.bfloat16)                      # [64, 1024]
    cc = np.ascontiguousarray(np.concatenate([gT, Wb, Vb], axis=1))
    return at, cc


def _prepare_in_maps(x, W_emb, b_emb, w_seg, b_seg):
    x = np.ascontiguousarray(np.asarray(x, dtype=np.float32))
    W_emb = np.asarray(W_emb, dtype=np.float32)
    b_emb = np.asarray(b_emb, dtype=np.float32)
    w_seg = np.asarray(w_seg, dtype=np.float32)
    b_seg = np.asarray(b_seg, dtype=np.float32)

    at, cc = _host_constants(W_emb, b_emb, w_seg, b_seg)

    in_maps = []
    for c in range(N_CORES):
        xs = x[c * B_LOC:(c + 1) * B_LOC].reshape(ROWS, F)
        xr = np.ascontiguousarray(
            xs.reshape(N_TILES, TILE_P, F).transpose(1, 0, 2).reshape(
                TILE_P, N_TILES * F)).astype(ml_dtypes.bfloat16)
        in_maps.append(
            {"xc": np.ascontiguousarray(
                np.concatenate([at, xr[:, 0:4 * F], xr], axis=1)),
             "cc": cc})
    return in_maps


def kernel(x, W_emb, b_emb, w_seg, b_seg):
    in_maps = _prepare_in_maps(x, W_emb, b_emb, w_seg, b_seg)

    global _NC_CACHE
    if _NC_CACHE is None:
        _NC_CACHE = _build_nc()

    res = run_bass_kernel_spmd(_NC_CACHE, in_maps,
                               core_ids=list(range(N_CORES)))
    out = np.concatenate(
        [np.asarray(res.results[c]["out"])
         .reshape(TILE_P, N_TILES, D).transpose(1, 0, 2)
         .reshape(B_LOC, S, D).astype(np.float32)
         for c in range(N_CORES)], axis=0)
    return out
